# revision 20
# baseline (speedup 1.0000x reference)
"""Trainium2 Bass kernel for nn_ModelNew_78847009620052 (dense_mlp).

Computes, for x [4096, 8192] and weight [8192, 8192]:
    out[b, 0] = 0.75 * sum_i x[b, i] * (sum_j weight[j, i])
(which equals 1.5 * sum(x @ W.T / 2, axis=1, keepdims=True)).

Sharding: column-shard the contraction dim IN=8192 into 8 chunks of 1024.
Core d receives x[:, d*1024:(d+1)*1024] and weight[:, d*1024:(d+1)*1024],
produces a partial [128, 32] result; host sums the 8 partials (after a
[128,32] -> [4096,1] reindex).

Per-core device algorithm (memory-bound: 48MB of input per core; the HW
sustains ~406 GB/s/core of HBM read when the pipeline is clean):
  Phase 1: stream weight rows; pre-accumulate on VectorE; reduce over the
           partition axis AND broadcast to 128 partitions on TensorE via
           matmul with an all-ones*0.75 stationary (scale folded in).
           Stream structure tuned for the serial tail between the last
           weight byte and the broadcast column sums being ready:
             - 2 head singles ([128,1024] 512KB DMAs): small first DMA
               so the first HBM byte lands ~1.2us earlier.
             - body of 1MB [128,2,1024] DMAs in groups [4]*7+[2] with
               VectorE tree pre-reduction (2 matmuls per group).
             - 2 tail singles, summed by ONE VectorE add, then one
               matmul pair; PSUM 'stop' on that pair.
           The PSUM->SBUF broadcast copy is split across ScalarE and
           VectorE (halves run in parallel, plain copies since the 0.75
           lives in the ones operand).
  Phase 2: stream 16 x DMAs of 1MB ([128,2,1024], two row-tiles each);
           VectorE does ONLY the products ([128,2048] per op, into SBUF);
           ScalarE reduces each [128,1024] half via
           activation(Copy, accum_out=...) into an SBUF [128,32] column.
           The [128,32] result is stored AS-IS; the host reindexes
           (out[c*128 + p] = O[p, c]).

Rationale (from NTFF traces): the kernel is pure HBM-roofline; all
engine work fits inside the DMA windows, so every revision targets the
serial tails (startup, w->x transition, post-stream drain) and keeps a
few microseconds of slack in every producer/consumer pair so one DMA
hiccup cannot re-gate the stream (the x-DMA issue is buffer-gated; with
8 x 1MB buffers the gating only engages ~6 tiles ahead of consumption).
(tensor_tensor_reduce would fuse phase 2 into one VectorE op, but that
opcode crashes the device on this HW/NRT path - validated by bisection.)
"""

import numpy as np

B, IN, HID = 4096, 8192, 8192
N_CORES = 8
CHUNK = IN // N_CORES          # 1024 columns per core
SCALE = 1.5 / 2.0              # 0.75, folded into the ones stationary
P = 128                        # partitions
W_TILES = HID // P             # 64 weight row-tiles per core
X_TILES = B // P               # 32 x row-tiles per core
XD = X_TILES // 2              # 16 x DMAs (two row-tiles each)

_compiled_nc = None


def _build_nc():
    import concourse.bass as bass
    import concourse.tile as tile
    from concourse import bacc, mybir

    f32 = mybir.dt.float32
    nc = bacc.Bacc(
        "TRN2",
        target_bir_lowering=False,
        debug=False,
        num_devices=N_CORES,
    )

    x_d = nc.dram_tensor("x", [B, CHUNK], f32, kind="ExternalInput")
    w_d = nc.dram_tensor("w", [HID, CHUNK], f32, kind="ExternalInput")
    out_d = nc.dram_tensor("out", [P, X_TILES], f32, kind="ExternalOutput")

    with tile.TileContext(nc) as tc:
        with (
            tc.tile_pool(name="wpool", bufs=9) as wpool,
            tc.tile_pool(name="xpool", bufs=6) as xpool,
            tc.tile_pool(name="const", bufs=1) as const,
            tc.tile_pool(name="psum", bufs=1, space="PSUM") as psum_pool,
        ):
            ones = const.tile([P, P], f32)
            nc.vector.memset(ones[:], SCALE)

            # Column sums land TWICE in PSUM ([P, 2, CHUNK], 4 banks) so
            # phase-2 muls can read a [128, 2048] broadcast operand straight
            # from PSUM - no PSUM->SBUF copy on the critical transition.
            psum_bc = psum_pool.tile([P, 2, CHUNK], f32, tag="psum_bc")

            def colsum_pair(src_ap, start, stop):
                for h in range(2):
                    nc.tensor.matmul(
                        psum_bc[:, 0, h * 512 : (h + 1) * 512],
                        ones[:],
                        src_ap[:, h * 512 : (h + 1) * 512],
                        start=start,
                        stop=stop,
                    )

            # --- Phase 1 head: two 512KB singles (fast first byte). ---
            row = 0
            for t in range(2):
                wt = wpool.tile([P, CHUNK], f32, tag="whead", bufs=2)
                nc.sync.dma_start(wt[:], w_d[row * P : (row + 1) * P, :])
                colsum_pair(wt, start=(t == 0), stop=False)
                row += 1

            # --- Phase 1 body: 1MB DMAs, grouped tree pre-reduction.
            # The trailing [2, 1] groups minimize the serial chain after
            # the final weight byte: in-tile add -> matmul pair (stop). ---
            GROUPS = [4, 4, 4, 4, 4, 4, 4, 2, 1]  # in 2-row-tile units
            assert 2 + sum(GROUPS) * 2 == W_TILES
            for gi, group in enumerate(GROUPS):
                last_group = gi == len(GROUPS) - 1
                wts = []
                for k in range(group):
                    wt = wpool.tile([P, 2, CHUNK], f32, tag="wtile")
                    src = w_d[(row + 2 * k) * P : (row + 2 * k + 2) * P, :]
                    nc.sync.dma_start(
                        wt[:], src.rearrange("(t p) c -> p t c", p=P)
                    )
                    nc.vector.tensor_add(
                        wt[:, 0, :], wt[:, 0, :], wt[:, 1, :]
                    )
                    wts.append(wt)
                row += 2 * group
                s = 1
                while s < group:
                    for k in range(0, group, 2 * s):
                        nc.vector.tensor_add(
                            wts[k][:, 0, :], wts[k][:, 0, :], wts[k + s][:, 0, :]
                        )
                    s *= 2
                colsum_pair(wts[0][:, 0, :], start=False, stop=last_group)
            assert row == W_TILES

            # --- Phase 2: x stream; VectorE products read the broadcast
            # column sums straight from PSUM; ScalarE reduces from SBUF.
            # The FIRST x pair is computed as two [128,1024] muls against
            # region 0 only, so it starts right after the stop-matmuls;
            # the region-1 dup copies run on VectorE right after it (off
            # the critical path), and all later muls are fused [128,2048].
            s_sbuf = const.tile([P, X_TILES], f32)
            scratch = const.tile([P, CHUNK], f32)
            wb_flat = psum_bc[:].rearrange("p t c -> p (t c)")
            for i in range(XD):
                xt = xpool.tile([P, 2, CHUNK], f32, tag="xtile")
                src = x_d[2 * i * P : (2 * i + 2) * P, :]
                nc.sync.dma_start(xt[:], src.rearrange("(t p) c -> p t c", p=P))
                prod = xpool.tile([P, 2, CHUNK], f32, tag="prod", bufs=5)
                if i == 0:
                    for h in range(2):
                        nc.vector.tensor_mul(
                            prod[:, h, :], xt[:, h, :], psum_bc[:, 0, :]
                        )
                    for h in range(2):
                        nc.vector.tensor_copy(
                            psum_bc[:, 1, h * 512 : (h + 1) * 512],
                            psum_bc[:, 0, h * 512 : (h + 1) * 512],
                        )
                else:
                    nc.vector.tensor_mul(
                        prod[:].rearrange("p t c -> p (t c)"),
                        xt[:].rearrange("p t c -> p (t c)"),
                        wb_flat,
                    )
                for h in range(2):
                    # The last 3 reduces run on VectorE (it finishes its
                    # muls ~4us before ScalarE's reduce chain drains;
                    # a 29/3 split lands both engines together).
                    if 2 * i + h >= X_TILES - 3:
                        nc.vector.reduce_sum(
                            s_sbuf[:, 2 * i + h : 2 * i + h + 1],
                            prod[:, h, :],
                            axis=mybir.AxisListType.X,
                        )
                    else:
                        nc.scalar.activation(
                            scratch[:],
                            prod[:, h, :],
                            mybir.ActivationFunctionType.Copy,
                            bias=0.0,
                            scale=1.0,
                            accum_out=s_sbuf[:, 2 * i + h : 2 * i + h + 1],
                        )

            # Store [128, 32] as-is: 128B contiguous run per partition.
            nc.sync.dma_start(out_d[:], s_sbuf[:])

    nc.compile()
    return nc


def _get_nc():
    global _compiled_nc
    if _compiled_nc is None:
        _compiled_nc = _build_nc()
    return _compiled_nc


def kernel(x: np.ndarray, weight: np.ndarray) -> np.ndarray:
    from concourse.bass_utils import run_bass_kernel_spmd

    x = np.asarray(x, dtype=np.float32)
    weight = np.asarray(weight, dtype=np.float32)
    assert x.shape == (B, IN) and weight.shape == (HID, IN)

    nc = _get_nc()
    in_maps = [
        {
            "x": np.ascontiguousarray(x[:, d * CHUNK : (d + 1) * CHUNK]),
            "w": np.ascontiguousarray(weight[:, d * CHUNK : (d + 1) * CHUNK]),
        }
        for d in range(N_CORES)
    ]
    res = run_bass_kernel_spmd(nc, in_maps, core_ids=list(range(N_CORES)))
    acc = np.zeros((B, 1), dtype=np.float64)
    for d in range(N_CORES):
        acc += res.results[d]["out"].T.reshape(B, 1).astype(np.float64)
    return acc.astype(np.float32)
